# revision 2
# baseline (speedup 1.0000x reference)
"""Chamfer distance kernel for Trainium2 (8 NeuronCores, SPMD).

Problem: B=4 batches, N=M=8192 points, D=3. Per batch:
    d2[n,m] = ||a_n - b_m||^2  (clamped at 0)
    out[b]  = mean_n(min_m d2) + mean_m(min_n d2)

Sharding: core c handles batch c//2, rows [h*4096,(h+1)*4096) of pc1 (h=c%2).
Each core computes, for its 4096x8192 block of the distance matrix:
  - rowmins: per-row min over all 8192 columns         -> [128, 32] fp32
  - colacc : per-column min over its 4096 rows (as a
             128-partition-wise partial min)            -> [128, 8192] fp16
Host combines: full col-min = min over partitions and over the 2 cores of a
batch; relu (= the reference's maximum(d2,0), which commutes with min) and
the tiny means run on host.

On-core pipeline per 128-row tile (32 tiles):
  PE    : 16 matmuls K=13 fp16 hi/lo-split -> psum = a2 - 2 a.b + b2 (fp32).
          The hi/lo split reproduces the fp32 product to ~1e-5 absolute at
          full fp16 PE rate; a2 rides two extra K rows so the escape needs
          no bias and no relu.
  ScalarE: pure Copy escape psum -> fp16 SBUF (1 elem/cycle/lane floor).
  VectorE: col-min fold (tensor_tensor min, 2x_1p) + row-min via ONE custom
          DVE op per tile: body=min(Src0,Src1) over the two 4096-col halves
          with accum=MIN -> rowmins[:,i]. The op ships a hand-written
          2X_1PORT uop program (pairs via SRC_*_HI) so the whole row-min
          costs ~2048 DVE cycles/tile instead of the stock tree's ~4100.

DVE/Act are near-balanced; a small slice of the escape is movable to DVE
via DVE_ESC if Act binds.  Accuracy: ~5e-05 max rel error vs fp32 reference.
"""

import numpy as np

B, N, M, D = 4, 8192, 8192, 3
NCORES = 8
NH = N // 2          # rows per core
NT = NH // 128       # 32 n-tiles of 128 rows
K = 13               # split-matmul contraction size (a2 folded in)
DVE_ESC = 0          # columns of each tile escaped by DVE instead of Act

_CACHE = {}


def _register_minmin_op():
    """Register the MINMIN2_ANT custom DVE op (min body + MIN accum) with a
    hand-written 2X_1PORT uop variant. Idempotent per process."""
    import concourse.dve_ops as dve_ops_mod
    from concourse.dve_spec import Spec, Src0, Src1, C1, minn, lower
    from concourse.dve_spec import AluOp as SAluOp
    from concourse.dve_uop import (
        DveOpSpec, UopConfig, UopDpConfig, InpSel, OutPath, OutSel,
        AluInp, DelayInp, Trigger,
    )

    NAME = "MINMIN2_ANT"
    if NAME in dve_ops_mod._SUB_OPCODE_FOR_NAME:
        for op in dve_ops_mod.OPS:
            if op.name == NAME:
                return op

    def _ref(in0, in1, s0, s1, imm2):
        b = np.minimum(in0.astype(np.float32), in1.astype(np.float32))
        acc = np.minimum(
            np.float32(s1),
            b.reshape(b.shape[0], -1).min(axis=-1, keepdims=True),
        ).astype(np.float32)
        return b, acc

    spec = Spec(body=minn(Src0, Src1), accum=SAluOp.MIN, accum_init=C1,
                reference=_ref)

    def _build_2x():
        """2X_1PORT program: two fp16 elements per cycle per lane.
        lanes: 0=SRC_0 1=SRC_1 2=SRC_0_HI 3=SRC_1_HI 4=CONST_1.
        s0: lo=min(S0,S1); s1: hi=min(S0H,S1H), lane5<-lo; s2: pair=min(hi,lo)
        s3: acc=min(acc,pair) [CURR_ALU_OUT flop]; s4-7 relay acc for readout.
        out passthrough of SRC_0/SRC_0_HI (scratch, matches 1x lower())."""
        MIN, BYP = SAluOp.MIN, SAluOp.BYPASS

        def dp_relay():
            d = UopDpConfig()
            d.enable_alu(BYP, AluInp.PREV_ALU_OUT, AluInp.PREV_ALU_OUT)
            d.alu_out_a_enable = 1
            return d

        # ---- steady state ----
        st = UopConfig()
        st.enable_input(InpSel.SRC_0, 1).enable_input(InpSel.SRC_1, 2)
        st.enable_input(InpSel.SRC_0_HI, 3).enable_input(InpSel.SRC_1_HI, 4)
        st.enable_input(InpSel.CONST_1, 5)
        s = st.datapath_config
        s[0] = (UopDpConfig()
                .enable_alu(MIN, AluInp.PREV_DELAY_0, AluInp.PREV_DELAY_1)
                .pass_through_delay(0, 2, 3))
        s[1] = (UopDpConfig()
                .enable_alu(MIN, AluInp.PREV_DELAY_2, AluInp.PREV_DELAY_3)
                .enable_delay_from_src(DelayInp.PREV_ALU_OUT, 5)
                .pass_through_delay(0, 2))
        s[2] = (UopDpConfig()
                .enable_alu(MIN, AluInp.PREV_ALU_OUT, AluInp.PREV_DELAY_5)
                .pass_through_delay(0, 2))
        s[3] = (UopDpConfig()
                .enable_alu(MIN, AluInp.CURR_ALU_OUT, AluInp.PREV_ALU_OUT)
                .pass_through_delay(0, 2))
        s[3].alu_out_a_enable = 1
        for i in range(4, 8):
            s[i] = dp_relay().pass_through_delay(0, 2)
        st.enable_output(OutSel.DELAY_0, OutPath.WR0_LO)
        st.enable_output(OutSel.DELAY_2, OutPath.WR0_HI)
        st.require_inp0 = 1
        st.require_inp1 = 1
        st.trigger = (Trigger.SRC_TENSOR_DONE, Trigger.NONE, Trigger.NONE)
        st.next_uop = (0, 0, 0)
        st.accum_enabled = 1
        st.repeat_count = 0

        # ---- seed: load accum_init (CONST_1) into stage-3 flop ----
        sd = UopConfig()
        sd.enable_input(InpSel.SRC_0, 1).enable_input(InpSel.SRC_1, 2)
        sd.enable_input(InpSel.SRC_0_HI, 3).enable_input(InpSel.SRC_1_HI, 4)
        sd.enable_input(InpSel.CONST_1, 5)
        d = sd.datapath_config
        for i in range(3):
            d[i] = (UopDpConfig()
                    .enable_alu(BYP, AluInp.PREV_ALU_OUT, AluInp.PREV_ALU_OUT)
                    .pass_through_delay(4))
        d[3] = UopDpConfig().enable_alu(BYP, AluInp.PREV_DELAY_4,
                                        AluInp.PREV_DELAY_4)
        d[3].alu_out_a_enable = 1
        for i in range(4, 8):
            d[i] = dp_relay()
        sd.require_inp0 = 0
        sd.require_inp1 = 0
        sd.trigger = (Trigger.COUNT, Trigger.NONE, Trigger.NONE)
        sd.repeat_count = 1
        sd.next_uop = (1, 0, 0)
        sd.accum_enabled = 1
        return [sd, st]

    row = max(dve_ops_mod._SUB_OPCODE_FOR_NAME.values()) + 1
    assert row < 0x20

    class _MinMinOp:
        name = NAME
        subdim = False

        def __init__(self):
            self.spec = spec
            self._cache = {}

        def compile(self, ver):
            if ver in self._cache:
                return self._cache[ver]
            r = DveOpSpec(
                name=NAME,
                opcode=row,
                uops=lower(spec, ver=ver),
                rd1_en=True,
                uops_2x=_build_2x() if ver == "v3" else None,
                perf_max=1 if ver == "v3" else 0,
            )
            self._cache[ver] = r
            return r

    op = _MinMinOp()
    dve_ops_mod.OPS.append(op)
    dve_ops_mod.CUSTOM_DVE_SPECS[NAME] = spec
    dve_ops_mod._SUB_OPCODE_FOR_NAME[NAME] = row
    return op


def _build(reps=1, tiny_out=False, use_2x=True, dve_esc=DVE_ESC,
           alloc_mode="stack"):
    """Build + compile the SPMD NEFF once per process.

    reps>1 repeats the main loop (identical results) — used only for
    slope-based execution timing; the product path uses reps=1."""
    import concourse.bacc as bacc
    import concourse.tile as tile
    import concourse.mybir as mybir

    minmin = _register_minmin_op()

    nc = bacc.Bacc("TRN2", target_bir_lowering=False, debug=False,
                   num_devices=NCORES)
    f16, f32 = mybir.dt.float16, mybir.dt.float32

    w_d = nc.dram_tensor("w", [K, NH], f16, kind="ExternalInput")
    bh_d = nc.dram_tensor("bh", [K, M], f16, kind="ExternalInput")
    colacc_shape = [128, 32] if tiny_out else [128, M]
    colacc_d = nc.dram_tensor("colacc", colacc_shape, f16,
                              kind="ExternalOutput")
    rowmins_d = nc.dram_tensor("rowmins", [128, NT], f32,
                               kind="ExternalOutput")

    tmin = mybir.AluOpType.min
    BIG = 3.0e38

    with tile.TileContext(nc, pool_alloc_mode=alloc_mode) as tc:
        with (
            tc.tile_pool(name="consts", bufs=1) as consts,
            tc.tile_pool(name="psum", bufs=2, space="PSUM") as psum_pool,
            tc.tile_pool(name="d2", bufs=2) as d2_pool,
            tc.tile_pool(name="scr", bufs=2) as scr_pool,
        ):
            w_sb = consts.tile([K, NH], f16)
            nc.sync.dma_start(out=w_sb, in_=w_d.ap())
            bh_sb = consts.tile([K, M], f16)
            nc.sync.dma_start(out=bh_sb, in_=bh_d.ap())

            colacc = consts.tile([128, M], f16)
            rowmins = consts.tile([128, NT], f32)

            for i in [t for _ in range(reps) for t in range(NT)]:
                d2row = d2_pool.tile([128, M], f16, tag="d2row")
                for q in range(4):
                    ps = psum_pool.tile([128, 2048], f32, tag="ps")
                    for jj in range(4):
                        j = q * 4 + jj
                        nc.tensor.matmul(
                            ps[:, jj * 512:(jj + 1) * 512],
                            w_sb[:, i * 128:(i + 1) * 128],
                            bh_sb[:, j * 512:(j + 1) * 512],
                            start=True, stop=True,
                        )
                    lo, hi = q * 2048, (q + 1) * 2048
                    split = 2048 - (dve_esc // 4)
                    if split > 0:
                        nc.scalar.activation(
                            out=d2row[:, lo:lo + split],
                            in_=ps[:, :split],
                            func=mybir.ActivationFunctionType.Copy,
                            bias=0.0, scale=1.0,
                        )
                    if split < 2048:
                        nc.vector.tensor_copy(
                            out=d2row[:, lo + split:hi],
                            in_=ps[:, split:],
                        )
                # direction-2: fold this row-block into the column-min accum
                if i == 0:
                    nc.vector.tensor_copy(out=colacc, in_=d2row)
                else:
                    nc.vector.tensor_tensor(out=colacc, in0=colacc,
                                            in1=d2row, op=tmin)
                # direction-1: fused pair-min + MIN-accum over the row
                scr = scr_pool.tile([128, M // 2], f16, tag="scr")
                inst = nc.vector._custom_dve(
                    minmin,
                    out=scr,
                    in0=d2row[:, :M // 2],
                    in1=d2row[:, M // 2:],
                    s1=BIG,
                    accum_out=rowmins[:, i:i + 1],
                )
                if use_2x:
                    inst.perf_max = 1

            if tiny_out:
                nc.sync.dma_start(out=colacc_d.ap(), in_=colacc[:, :32])
            else:
                nc.sync.dma_start(out=colacc_d.ap(), in_=colacc)
            nc.sync.dma_start(out=rowmins_d.ap(), in_=rowmins)

    nc.compile()
    return nc


def _prep_inputs(pc1, pc2):
    """Host-side: build per-core fp16 hi/lo split operands (tiny arrays)."""
    in_maps = []
    for c in range(NCORES):
        b, h = divmod(c, 2)
        a = np.asarray(pc1[b][h * NH:(h + 1) * NH], dtype=np.float32)  # [NH,3]
        bb = np.asarray(pc2[b], dtype=np.float32)                      # [M,3]

        ah = a.astype(np.float16)
        al = (a - ah.astype(np.float32)).astype(np.float16)
        a2 = np.square(a.astype(np.float64)).sum(-1)                   # [NH]
        a2h = a2.astype(np.float16)
        a2l = (a2 - a2h.astype(np.float64)).astype(np.float16)
        w = np.empty((K, NH), dtype=np.float16)
        w[0:3] = (ah.T * np.float16(-2))
        w[3:6] = (al.T * np.float16(-2))
        w[6:9] = (ah.T * np.float16(-2))
        w[9] = np.float16(1.0)
        w[10] = np.float16(1.0)
        w[11] = a2h
        w[12] = a2l

        bhh = bb.astype(np.float16)
        bl = (bb - bhh.astype(np.float32)).astype(np.float16)
        b2 = np.square(bb.astype(np.float64)).sum(-1)                  # [M]
        b2h = b2.astype(np.float16)
        b2l = (b2 - b2h.astype(np.float64)).astype(np.float16)
        bh = np.empty((K, M), dtype=np.float16)
        bh[0:3] = bhh.T
        bh[3:6] = bhh.T
        bh[6:9] = bl.T
        bh[9] = b2h
        bh[10] = b2l
        bh[11] = np.float16(1.0)
        bh[12] = np.float16(1.0)

        in_maps.append({"w": w, "bh": bh})
    return in_maps


def _run(in_maps, trace=False):
    from concourse.bass_utils import run_bass_kernel_spmd
    if "nc" not in _CACHE:
        _CACHE["nc"] = _build()
    return run_bass_kernel_spmd(_CACHE["nc"], in_maps,
                                core_ids=list(range(NCORES)), trace=trace)


def kernel(pc1, pc2, _trace=False):
    pc1 = np.asarray(pc1, dtype=np.float32)
    pc2 = np.asarray(pc2, dtype=np.float32)
    res = _run(_prep_inputs(pc1, pc2), trace=_trace)

    out = np.empty((B,), dtype=np.float32)
    for b in range(B):
        r0, r1 = res.results[2 * b], res.results[2 * b + 1]
        colmin = np.minimum(
            r0["colacc"].astype(np.float32).min(axis=0),
            r1["colacc"].astype(np.float32).min(axis=0),
        )                                                              # [M]
        term2 = np.maximum(colmin, 0.0).mean(dtype=np.float64)
        rowmins = np.concatenate([r0["rowmins"].ravel(),
                                  r1["rowmins"].ravel()])
        term1 = np.maximum(rowmins, 0.0).mean(dtype=np.float64)
        out[b] = np.float32(term1 + term2)
    kernel._last_results = res
    return out


# revision 6
# speedup vs baseline: 1.9715x; 1.9715x over previous
"""Chamfer distance kernel for Trainium2 (8 NeuronCores, SPMD).

Problem: B=4 batches, N=M=8192 points, D=3. Per batch:
    d2[n,m] = ||a_n - b_m||^2  (clamped at 0)
    out[b]  = mean_n(min_m d2) + mean_m(min_n d2)

Sharding: core c handles batch c//2, rows [h*4096,(h+1)*4096) of pc1 (h=c%2).
Each core computes, for its 4096x8192 block of the distance matrix:
  - rowmins: per-row min over all 8192 columns         -> [128, 32] fp32
  - colacc : per-column min over its 4096 rows (as a
             128-partition-wise partial min)            -> [128, 8192] fp16
Host combines: full col-min = min over partitions and over the 2 cores of a
batch; relu (= the reference's maximum(d2,0), which commutes with min) and
the tiny means run on host.

On-core pipeline per 128-row tile (32 tiles):
  PE    : 16 matmuls K=13 fp16 hi/lo-split -> psum = a2 - 2 a.b + b2 (fp32).
          The hi/lo split reproduces the fp32 product to ~1e-5 absolute at
          full fp16 PE rate; a2 rides two extra K rows so the escape needs
          no bias and no relu.
  ScalarE: pure Copy escape psum -> fp16 SBUF (1 elem/cycle/lane floor).
  VectorE: col-min fold (tensor_tensor min, 2x_1p) + row-min binary tree
          (7 tensor_tensor min levels + one tensor_reduce), all flat 2D APs.

Relative to the previous version this folds a2 into the matmul (dropping the
per-partition bias AP from every escape), uses flat 2D access patterns
everywhere (no [128,2,w] pairing), and moves the reference's max(d2,0) to the
host (relu commutes with min), keeping the escape a plain Relu activation.
Measured by reps-slope on HW: ~143 us/pass vs ~677 us for the previous
kernel in the same session (4.7x). A hand-written 2X_1PORT custom-DVE
fused rowmin op (rowmin="custom") measures 4780 ns/tile vs the tree's
4466 and is kept for reference. Accuracy: ~5e-05 max rel error.
"""

import numpy as np

B, N, M, D = 4, 8192, 8192, 3
NCORES = 8
NH = N // 2          # rows per core
NT = NH // 128       # 32 n-tiles of 128 rows
K = 13               # split-matmul contraction size (a2 folded in)
DVE_ESC = 0          # columns of each tile escaped by DVE instead of Act

_CACHE = {}


def _register_minmin_op():
    """Register the MINMIN2_ANT custom DVE op (min body + MIN accum) with a
    hand-written 2X_1PORT uop variant. Idempotent per process."""
    import concourse.dve_ops as dve_ops_mod
    from concourse.dve_spec import Spec, Src0, Src1, C1, minn, lower
    from concourse.dve_spec import AluOp as SAluOp
    from concourse.dve_uop import (
        DveOpSpec, UopConfig, UopDpConfig, InpSel, OutPath, OutSel,
        AluInp, DelayInp, Trigger,
    )

    NAME = "MINMIN2_ANT"
    if NAME in dve_ops_mod._SUB_OPCODE_FOR_NAME:
        for op in dve_ops_mod.OPS:
            if op.name == NAME:
                return op

    def _ref(in0, in1, s0, s1, imm2):
        b = np.minimum(in0.astype(np.float32), in1.astype(np.float32))
        acc = np.minimum(
            np.float32(s1),
            b.reshape(b.shape[0], -1).min(axis=-1, keepdims=True),
        ).astype(np.float32)
        return b, acc

    spec = Spec(body=minn(Src0, Src1), accum=SAluOp.MIN, accum_init=C1,
                reference=_ref)

    def _build_2x():
        """2X_1PORT program: two fp16 elements per cycle per lane.
        lanes: 0=SRC_0 1=SRC_1 2=SRC_0_HI 3=SRC_1_HI 4=CONST_1.
        s0: lo=min(S0,S1); s1: hi=min(S0H,S1H), lane5<-lo; s2: pair=min(hi,lo)
        s3: acc=min(acc,pair) [CURR_ALU_OUT flop]; s4-7 relay acc for readout.
        out passthrough of SRC_0/SRC_0_HI (scratch, matches 1x lower())."""
        MIN, BYP = SAluOp.MIN, SAluOp.BYPASS

        def dp_relay():
            d = UopDpConfig()
            d.enable_alu(BYP, AluInp.PREV_ALU_OUT, AluInp.PREV_ALU_OUT)
            d.alu_out_a_enable = 1
            return d

        # ---- steady state ----
        st = UopConfig()
        st.enable_input(InpSel.SRC_0, 1).enable_input(InpSel.SRC_1, 2)
        st.enable_input(InpSel.SRC_0_HI, 3).enable_input(InpSel.SRC_1_HI, 4)
        st.enable_input(InpSel.CONST_1, 5)
        s = st.datapath_config
        s[0] = (UopDpConfig()
                .enable_alu(MIN, AluInp.PREV_DELAY_0, AluInp.PREV_DELAY_1)
                .pass_through_delay(0, 2, 3))
        s[1] = (UopDpConfig()
                .enable_alu(MIN, AluInp.PREV_DELAY_2, AluInp.PREV_DELAY_3)
                .enable_delay_from_src(DelayInp.PREV_ALU_OUT, 5)
                .pass_through_delay(0, 2))
        s[2] = (UopDpConfig()
                .enable_alu(MIN, AluInp.PREV_ALU_OUT, AluInp.PREV_DELAY_5)
                .pass_through_delay(0, 2))
        s[3] = (UopDpConfig()
                .enable_alu(MIN, AluInp.CURR_ALU_OUT, AluInp.PREV_ALU_OUT)
                .pass_through_delay(0, 2))
        s[3].alu_out_a_enable = 1
        for i in range(4, 8):
            s[i] = dp_relay().pass_through_delay(0, 2)
        st.enable_output(OutSel.DELAY_0, OutPath.WR0_LO)
        st.enable_output(OutSel.DELAY_2, OutPath.WR0_HI)
        st.require_inp0 = 1
        st.require_inp1 = 1
        st.trigger = (Trigger.SRC_TENSOR_DONE, Trigger.NONE, Trigger.NONE)
        st.next_uop = (0, 0, 0)
        st.accum_enabled = 1
        st.repeat_count = 0

        # ---- seed: load accum_init (CONST_1) into stage-3 flop ----
        sd = UopConfig()
        sd.enable_input(InpSel.SRC_0, 1).enable_input(InpSel.SRC_1, 2)
        sd.enable_input(InpSel.SRC_0_HI, 3).enable_input(InpSel.SRC_1_HI, 4)
        sd.enable_input(InpSel.CONST_1, 5)
        d = sd.datapath_config
        for i in range(3):
            d[i] = (UopDpConfig()
                    .enable_alu(BYP, AluInp.PREV_ALU_OUT, AluInp.PREV_ALU_OUT)
                    .pass_through_delay(4))
        d[3] = UopDpConfig().enable_alu(BYP, AluInp.PREV_DELAY_4,
                                        AluInp.PREV_DELAY_4)
        d[3].alu_out_a_enable = 1
        for i in range(4, 8):
            d[i] = dp_relay()
        sd.require_inp0 = 0
        sd.require_inp1 = 0
        sd.trigger = (Trigger.COUNT, Trigger.NONE, Trigger.NONE)
        sd.repeat_count = 1
        sd.next_uop = (1, 0, 0)
        sd.accum_enabled = 1
        return [sd, st]

    row = max(dve_ops_mod._SUB_OPCODE_FOR_NAME.values()) + 1
    assert row < 0x20

    class _MinMinOp:
        name = NAME
        subdim = False

        def __init__(self):
            self.spec = spec
            self._cache = {}

        def compile(self, ver):
            if ver in self._cache:
                return self._cache[ver]
            r = DveOpSpec(
                name=NAME,
                opcode=row,
                uops=lower(spec, ver=ver),
                rd1_en=True,
                uops_2x=_build_2x() if ver == "v3" else None,
                perf_max=1 if ver == "v3" else 0,
            )
            self._cache[ver] = r
            return r

    op = _MinMinOp()
    dve_ops_mod.OPS.append(op)
    dve_ops_mod.CUSTOM_DVE_SPECS[NAME] = spec
    dve_ops_mod._SUB_OPCODE_FOR_NAME[NAME] = row
    return op


def _build(reps=1, tiny_out=False, use_2x=True, dve_esc=DVE_ESC,
           act_func="relu", rowmin="tree", alloc_mode="stack"):
    """Build + compile the SPMD NEFF once per process.

    reps>1 repeats the main loop (identical results) — used only for
    slope-based execution timing; the product path uses reps=1."""
    import concourse.bacc as bacc
    import concourse.tile as tile
    import concourse.mybir as mybir

    minmin = _register_minmin_op()

    nc = bacc.Bacc("TRN2", target_bir_lowering=False, debug=False,
                   num_devices=NCORES)
    f16, f32 = mybir.dt.float16, mybir.dt.float32

    w_d = nc.dram_tensor("w", [K, NH], f16, kind="ExternalInput")
    bh_d = nc.dram_tensor("bh", [K, M], f16, kind="ExternalInput")
    colacc_shape = [128, 32] if tiny_out else [128, M]
    colacc_d = nc.dram_tensor("colacc", colacc_shape, f16,
                              kind="ExternalOutput")
    rowmins_d = nc.dram_tensor("rowmins", [128, NT], f32,
                               kind="ExternalOutput")

    tmin = mybir.AluOpType.min
    BIG = 3.0e38

    with tile.TileContext(nc, pool_alloc_mode=alloc_mode) as tc:
        with (
            tc.tile_pool(name="consts", bufs=1) as consts,
            tc.tile_pool(name="psum", bufs=2, space="PSUM") as psum_pool,
            tc.tile_pool(name="d2", bufs=2) as d2_pool,
            tc.tile_pool(name="scr", bufs=2) as scr_pool,
        ):
            w_sb = consts.tile([K, NH], f16)
            nc.sync.dma_start(out=w_sb, in_=w_d.ap())
            bh_sb = consts.tile([K, M], f16)
            nc.sync.dma_start(out=bh_sb, in_=bh_d.ap())

            colacc = consts.tile([128, M], f16)
            rowmins = consts.tile([128, NT], f32)

            for i in [t for _ in range(reps) for t in range(NT)]:
                d2row = d2_pool.tile([128, M], f16, tag="d2row")
                for q in range(4):
                    ps = psum_pool.tile([128, 2048], f32, tag="ps")
                    for jj in range(4):
                        j = q * 4 + jj
                        nc.tensor.matmul(
                            ps[:, jj * 512:(jj + 1) * 512],
                            w_sb[:, i * 128:(i + 1) * 128],
                            bh_sb[:, j * 512:(j + 1) * 512],
                            start=True, stop=True,
                        )
                    lo, hi = q * 2048, (q + 1) * 2048
                    split = 2048 - (dve_esc // 4)
                    if split > 0:
                        nc.scalar.activation(
                            out=d2row[:, lo:lo + split],
                            in_=ps[:, :split],
                            func=(mybir.ActivationFunctionType.Relu
                                  if act_func == "relu" else
                                  mybir.ActivationFunctionType.Copy),
                            bias=0.0, scale=1.0,
                        )
                    if split < 2048:
                        nc.vector.tensor_copy(
                            out=d2row[:, lo + split:hi],
                            in_=ps[:, split:],
                        )
                # direction-2: fold this row-block into the column-min accum
                if i == 0:
                    nc.vector.tensor_copy(out=colacc, in_=d2row)
                else:
                    nc.vector.tensor_tensor(out=colacc, in0=colacc,
                                            in1=d2row, op=tmin)
                # direction-1: fused pair-min + MIN-accum over the row
                if rowmin == "custom":
                    scr = scr_pool.tile([128, M // 2], f16, tag="scr")
                    inst = nc.vector._custom_dve(
                        minmin,
                        out=scr,
                        in0=d2row[:, :M // 2],
                        in1=d2row[:, M // 2:],
                        s1=BIG,
                        accum_out=rowmins[:, i:i + 1],
                    )
                    if use_2x:
                        inst.perf_max = 1
                elif rowmin == "tree":
                    tr = scr_pool.tile([128, M // 2], f16, tag="scr")
                    nc.vector.tensor_tensor(out=tr, in0=d2row[:, :4096],
                                            in1=d2row[:, 4096:], op=tmin)
                    nc.vector.tensor_tensor(out=tr[:, :2048], in0=tr[:, :2048],
                                            in1=tr[:, 2048:4096], op=tmin)
                    nc.vector.tensor_tensor(out=tr[:, :1024], in0=tr[:, :1024],
                                            in1=tr[:, 1024:2048], op=tmin)
                    nc.vector.tensor_tensor(out=tr[:, :512], in0=tr[:, :512],
                                            in1=tr[:, 512:1024], op=tmin)
                    nc.vector.tensor_tensor(out=tr[:, :256], in0=tr[:, :256],
                                            in1=tr[:, 256:512], op=tmin)
                    nc.vector.tensor_tensor(out=tr[:, :128], in0=tr[:, :128],
                                            in1=tr[:, 128:256], op=tmin)
                    nc.vector.tensor_tensor(out=tr[:, :64], in0=tr[:, :64],
                                            in1=tr[:, 64:128], op=tmin)
                    nc.vector.tensor_reduce(out=rowmins[:, i:i + 1],
                                            in_=tr[:, :64],
                                            axis=mybir.AxisListType.X, op=tmin)
                elif rowmin == "none":
                    if i == 0:
                        nc.gpsimd.memset(rowmins, 0.0)

            if tiny_out:
                nc.sync.dma_start(out=colacc_d.ap(), in_=colacc[:, :32])
            else:
                nc.sync.dma_start(out=colacc_d.ap(), in_=colacc)
            nc.sync.dma_start(out=rowmins_d.ap(), in_=rowmins)

    nc.compile()
    return nc


def _prep_inputs(pc1, pc2):
    """Host-side: build per-core fp16 hi/lo split operands (tiny arrays)."""
    in_maps = []
    for c in range(NCORES):
        b, h = divmod(c, 2)
        a = np.asarray(pc1[b][h * NH:(h + 1) * NH], dtype=np.float32)  # [NH,3]
        bb = np.asarray(pc2[b], dtype=np.float32)                      # [M,3]

        ah = a.astype(np.float16)
        al = (a - ah.astype(np.float32)).astype(np.float16)
        a2 = np.square(a.astype(np.float64)).sum(-1)                   # [NH]
        a2h = a2.astype(np.float16)
        a2l = (a2 - a2h.astype(np.float64)).astype(np.float16)
        w = np.empty((K, NH), dtype=np.float16)
        w[0:3] = (ah.T * np.float16(-2))
        w[3:6] = (al.T * np.float16(-2))
        w[6:9] = (ah.T * np.float16(-2))
        w[9] = np.float16(1.0)
        w[10] = np.float16(1.0)
        w[11] = a2h
        w[12] = a2l

        bhh = bb.astype(np.float16)
        bl = (bb - bhh.astype(np.float32)).astype(np.float16)
        b2 = np.square(bb.astype(np.float64)).sum(-1)                  # [M]
        b2h = b2.astype(np.float16)
        b2l = (b2 - b2h.astype(np.float64)).astype(np.float16)
        bh = np.empty((K, M), dtype=np.float16)
        bh[0:3] = bhh.T
        bh[3:6] = bhh.T
        bh[6:9] = bl.T
        bh[9] = b2h
        bh[10] = b2l
        bh[11] = np.float16(1.0)
        bh[12] = np.float16(1.0)

        in_maps.append({"w": w, "bh": bh})
    return in_maps


def _run(in_maps, trace=False):
    from concourse.bass_utils import run_bass_kernel_spmd
    if "nc" not in _CACHE:
        _CACHE["nc"] = _build()
    return run_bass_kernel_spmd(_CACHE["nc"], in_maps,
                                core_ids=list(range(NCORES)), trace=trace)


def kernel(pc1, pc2, _trace=False):
    pc1 = np.asarray(pc1, dtype=np.float32)
    pc2 = np.asarray(pc2, dtype=np.float32)
    res = _run(_prep_inputs(pc1, pc2), trace=_trace)

    out = np.empty((B,), dtype=np.float32)
    for b in range(B):
        r0, r1 = res.results[2 * b], res.results[2 * b + 1]
        colmin = np.minimum(
            r0["colacc"].astype(np.float32).min(axis=0),
            r1["colacc"].astype(np.float32).min(axis=0),
        )                                                              # [M]
        term2 = np.maximum(colmin, 0.0).mean(dtype=np.float64)
        rowmins = np.concatenate([r0["rowmins"].ravel(),
                                  r1["rowmins"].ravel()])
        term1 = np.maximum(rowmins, 0.0).mean(dtype=np.float64)
        out[b] = np.float32(term1 + term2)
    kernel._last_results = res
    return out
